# revision 1
# baseline (speedup 1.0000x reference)
"""Trainium2 Bass kernel for CohereAttention (B=2, S=2048, H=4096, 32Q/8KV heads, D=128).

Sharding: 8 cores = 2 batch groups x 4 tensor-parallel (head) ranks.
Core c: batch b = c // 4, tp rank t = c % 4.
  - owns q-heads [8t, 8t+8), kv-heads [2t, 2t+2) (GQA-aligned), w_o col slice
    [1024t, 1024(t+1)).
  - per-head attention output (transposed [d, s]) is AllGather'd across the 4
    ranks of the batch group, chunk-by-chunk (overlaps attention compute);
    o_proj then computes its 1024-column output slice with no all-reduce.

All matmuls run as float32r (TF32-like, full PE rate at moving-dim >= 256) on
fp32 data. RoPE is restructured host-side: q/k weight columns are permuted to
"neox" halves (evens then odds) so the on-device rotation is two block copies
plus elementwise ops against duplicated cos/sin tables.

Attention is computed in transposed score layout sT[k, q] so the exp'd scores
feed PV matmuls directly (no PE transposes); the softmax denominator comes from
a ones-vector matmul accumulated alongside PV, and normalization happens during
PSUM eviction.
"""

import numpy as np

import concourse.bass as bass
import concourse.mybir as mybir
from concourse import bacc
import concourse.tile as tile
from concourse.bass_utils import run_bass_kernel_spmd

# Problem constants (fixed by the task).
B, S, H = 2, 2048, 4096
NQ, NKV, D = 32, 8, 128
THETA = 10000.0
NCORES = 8
TP = 4                      # head-parallel group size
QH = NQ // TP               # 8 q heads per core
KH = NKV // TP              # 2 kv heads per core
REP = NQ // NKV             # 4
SCALE = float(D) ** -0.5
QC = QH * D                 # 1024 local q cols
KC = KH * D                 # 256 local k cols
OC = H // TP                # 1024 output cols per core
P = 128
KT = H // P                 # 32 contraction tiles for projections
AKT = NQ * D // P           # 32 contraction tiles for o_proj
NSB = S // P                # 16 seq blocks
SG = 512
NSG = S // SG               # 4 seq groups
F32 = mybir.dt.float32
F32R = mybir.dt.float32r
NCT = (QC + KC) // P        # 10 q/k col-tiles per core
RG = [[0, 1, 2, 3], [4, 5, 6, 7]]

Exp = mybir.ActivationFunctionType.Exp


def _r(ap):
    return ap.bitcast(F32R)


def build_program(no_collective=False, phase3_reads_att=False):
    """Emit the SPMD Bass program (same program for all 8 cores)."""
    nc = bacc.Bacc('TRN2', target_bir_lowering=False, debug=False, num_devices=NCORES)

    hidT = nc.dram_tensor("hidT", [H, S], F32, kind="ExternalInput")
    wqk = nc.dram_tensor("wqk", [H, QC + KC], F32, kind="ExternalInput")
    wv = nc.dram_tensor("wv", [H, KC], F32, kind="ExternalInput")
    wo = nc.dram_tensor("wo", [NQ * D, OC], F32, kind="ExternalInput")
    cosf = nc.dram_tensor("cosf", [P, S], F32, kind="ExternalInput")
    sins = nc.dram_tensor("sins", [P, S], F32, kind="ExternalInput")
    out = nc.dram_tensor("out", [S, OC], F32, kind="ExternalOutput")

    qkT_d = nc.dram_tensor("qkT_d", [QC + KC, S], F32)       # roped qT/kT
    v_d = nc.dram_tensor("v_d", [S, KC], F32)                # v natural
    att_d = nc.dram_tensor("att_d", [QH, P, S], F32)         # local attnT
    gath_d = nc.dram_tensor("gath_d", [QH, TP * P, S], F32)

    wqk_t = wqk.rearrange("(ko p) c -> p ko c", p=P)
    wv_t = wv.rearrange("(ko p) c -> p ko c", p=P)
    wo_t = wo.rearrange("(ko p) c -> p ko c", p=P)

    with tile.TileContext(nc) as tc:
        # ---------------- Phase 1: qkv projection + RoPE ----------------
        # Contraction (H=4096) is split in two halves so only half of
        # hidT's columns for the current seq half must be SBUF-resident;
        # partial products are accumulated in SBUF (acc/v_sb) across halves.
        with tc.tile_pool(name="ph1_hid", bufs=1) as hidp, \
             tc.tile_pool(name="ph1_w", bufs=2) as wp, \
             tc.tile_pool(name="ph1_wv", bufs=2) as wvp, \
             tc.tile_pool(name="ph1_cs", bufs=1) as csp, \
             tc.tile_pool(name="ph1_acc", bufs=1) as accp, \
             tc.tile_pool(name="ph1_rope", bufs=2) as rp, \
             tc.tile_pool(name="ph1_vsb", bufs=1) as vsp, \
             tc.tile_pool(name="ph1_ps", bufs=2, space="PSUM") as pp, \
             tc.tile_pool(name="ph1_psv", bufs=2, space="PSUM") as ppv:
            KH2 = KT // 2   # 16 k-tiles per contraction half
            for sg in range(2):          # seq halves of 1024
                sgs = slice(sg * 1024, (sg + 1) * 1024)
                cos_sb = csp.tile([P, 1024], F32, tag="cos")
                sin_sb = csp.tile([P, 1024], F32, tag="sin")
                nc.sync.dma_start(out=cos_sb[:], in_=cosf[:, sgs])
                nc.sync.dma_start(out=sin_sb[:], in_=sins[:, sgs])
                v_sb = vsp.tile([P, 8, KC], F32, tag="vacc")
                acc = accp.tile([P, NCT, 1024], F32, tag="acc")
                for kh in range(2):      # contraction halves (16 k-tiles)
                    hid_sb = hidp.tile([P, KH2, 1024], F32R, tag="hid")
                    for kt in range(KH2):
                        nc.sync.dma_start(
                            out=hid_sb[:, kt, :],
                            in_=hidT[(kh * KH2 + kt) * P:(kh * KH2 + kt + 1) * P, sgs].bitcast(F32R),
                        )
                    # q/k col-tiles: out[M=col, N=seq] = wqk.T @ hidT
                    for ct in range(NCT):
                        w_sb = wp.tile([P, KH2, P], F32R, tag="wqk")
                        for kt in range(KH2):
                            nc.sync.dma_start(
                                out=w_sb[:, kt, :],
                                in_=wqk_t[:, kh * KH2 + kt, ct * P:(ct + 1) * P].bitcast(F32R),
                            )
                        ps = pp.tile([P, 1024], F32, tag="ps", name=f"ps_{sg}_{kh}_{ct}")
                        for kt in range(KH2):
                            for nn in range(2):
                                nc.tensor.matmul(
                                    ps[:, nn * 512:(nn + 1) * 512],
                                    w_sb[:, kt, :],
                                    hid_sb[:, kt, nn * 512:(nn + 1) * 512],
                                    start=(kt == 0), stop=(kt == KH2 - 1),
                                )
                        for nn in range(2):
                            nsl = slice(nn * 512, (nn + 1) * 512)
                            if kh == 0:
                                nc.scalar.copy(acc[:, ct, nsl], ps[:, nsl])
                            else:
                                nc.vector.tensor_add(
                                    acc[:, ct, nsl], acc[:, ct, nsl], ps[:, nsl]
                                )
                    # v projection: out[M=seq, N=vcols] = hid @ wv
                    wv_sb = wvp.tile([P, KH2, KC], F32R, tag="wv")
                    for kt in range(KH2):
                        nc.sync.dma_start(
                            out=wv_sb[:, kt, :], in_=wv_t[:, kh * KH2 + kt, :].bitcast(F32R)
                        )
                    for sbl in range(8):
                        psv = ppv.tile(
                            [P, KC], F32, tag="psv", name=f"psv_{sg}_{kh}_{sbl}"
                        )
                        for kt in range(KH2):
                            nc.tensor.matmul(
                                psv[:],
                                hid_sb[:, kt, sbl * P:(sbl + 1) * P],
                                wv_sb[:, kt, :],
                                start=(kt == 0), stop=(kt == KH2 - 1),
                            )
                        if kh == 0:
                            nc.scalar.copy(v_sb[:, sbl, :], psv[:])
                        else:
                            nc.vector.tensor_add(v_sb[:, sbl, :], v_sb[:, sbl, :], psv[:])
                # RoPE on q/k (each col-tile is one whole head) + store
                for ct in range(NCT):
                    x = acc[:, ct, :]
                    tmp = rp.tile([P, 1024], F32, tag="tmp")
                    # rot(x) = [-x2; x1] (sign folded into sins rows)
                    nc.vector.tensor_copy(tmp[0:64, :], x[64:128, :])
                    nc.vector.tensor_copy(tmp[64:128, :], x[0:64, :])
                    t1 = rp.tile([P, 1024], F32, tag="t1")
                    nc.vector.tensor_mul(t1[:], x, cos_sb[:])
                    nc.vector.tensor_mul(tmp[:], tmp[:], sin_sb[:])
                    nc.vector.tensor_add(t1[:], t1[:], tmp[:])
                    nc.sync.dma_start(out=qkT_d[ct * P:(ct + 1) * P, sgs], in_=t1[:])
                for sbl in range(8):
                    nc.sync.dma_start(
                        out=v_d[sg * 1024 + sbl * P:sg * 1024 + (sbl + 1) * P, :],
                        in_=v_sb[:, sbl, :],
                    )

        # ---------------- Phase 2: attention (transposed scores) ---------
        with tc.tile_pool(name="ph2_kv", bufs=1) as kvp, \
             tc.tile_pool(name="ph2_q", bufs=2) as qp, \
             tc.tile_pool(name="ph2_p", bufs=6) as ppl, \
             tc.tile_pool(name="ph2_o", bufs=2) as op, \
             tc.tile_pool(name="ph2_c", bufs=1) as cp, \
             tc.tile_pool(name="ph2_ps", bufs=2, space="PSUM") as sp, \
             tc.tile_pool(name="ph2_pacc", bufs=2, space="PSUM") as ap_, \
             tc.tile_pool(name="ph2_pden", bufs=2, space="PSUM") as dp, \
             tc.tile_pool(name="ph2_pbc", bufs=1, space="PSUM") as bp:
            ones_f = cp.tile([P, 1], F32, tag="ones_f")
            nc.vector.memset(ones_f[:], 1.0)
            ones_sb = cp.tile([P, 1], F32R, tag="ones")
            nc.sync.dma_start(out=ones_sb[:], in_=ones_f[:].bitcast(F32R))
            ones_row = cp.tile([1, P], F32, tag="onesr")
            nc.vector.memset(ones_row[:], 1.0)
            for kv in range(KH):
                kT_sb = kvp.tile([P, S], F32R, tag="kT")
                nc.sync.dma_start(out=kT_sb[:], in_=qkT_d[QC + kv * P:QC + (kv + 1) * P, :].bitcast(F32R))
                vn_sb = kvp.tile([P, NSB, P], F32R, tag="vn")
                vd_r = v_d.rearrange("(nb p) c -> p nb c", p=P)
                nc.sync.dma_start(out=vn_sb[:], in_=vd_r[:, :, kv * P:(kv + 1) * P].bitcast(F32R))
                for qi in range(REP):
                    qh = kv * REP + qi
                    qT_sb = qp.tile([P, S], F32R, tag="qT")
                    nc.sync.dma_start(out=qT_sb[:], in_=qkT_d[qh * P:(qh + 1) * P, :].bitcast(F32R))
                    for j in range(NSG):
                        ncb = 4 * j + 4
                        att_ps = ap_.tile([P, SG], F32, tag="att", name=f"att_{qh}_{j}")
                        den_ps = dp.tile([1, SG], F32, tag="den", name=f"den_{qh}_{j}")
                        qs = slice(j * SG, (j + 1) * SG)
                        for c in range(ncb):
                            s_ps = sp.tile([P, SG], F32, tag="s", name=f"s_{qh}_{j}_{c}")
                            nc.tensor.matmul(
                                s_ps[:],
                                kT_sb[:, c * P:(c + 1) * P],
                                qT_sb[:, qs],
                                start=True, stop=True,
                            )
                            p_sb = ppl.tile([P, SG], F32R, tag="p", name=f"p_{qh}_{j}_{c}")
                            nc.scalar.activation(p_sb[:], s_ps[:], Exp, scale=SCALE)
                            if c >= 4 * j:
                                # zero p where k > q (causal), diagonal chunk
                                r = c - 4 * j
                                nc.gpsimd.affine_select(
                                    out=p_sb[:], in_=p_sb[:],
                                    compare_op=mybir.AluOpType.is_ge,
                                    fill=0.0, base=-(P * r),
                                    pattern=[[1, SG]], channel_multiplier=-1,
                                )
                            nc.tensor.matmul(
                                den_ps[:], ones_sb[:], p_sb[:],
                                start=(c == 0), stop=(c == ncb - 1),
                            )
                            nc.tensor.matmul(
                                att_ps[:], vn_sb[:, c, :], p_sb[:],
                                start=(c == 0), stop=(c == ncb - 1),
                            )
                        rinv = op.tile([1, SG], F32, tag="rinv")
                        nc.vector.reciprocal(rinv[:], den_ps[:])
                        rb_ps = bp.tile([P, SG], F32, tag="rb", name=f"rb_{qh}_{j}")
                        nc.tensor.matmul(
                            rb_ps[:], ones_row[:], rinv[:],
                            start=True, stop=True,
                        )
                        rb_sb = op.tile([P, SG], F32, tag="rb_sb")
                        nc.scalar.copy(rb_sb[:], rb_ps[:])
                        att_sb = op.tile([P, SG], F32, tag="att_sb")
                        nc.vector.tensor_mul(att_sb[:], att_ps[:], rb_sb[:])
                        nc.sync.dma_start(out=att_d[qh, :, qs], in_=att_sb[:])
                    if no_collective:
                        nc.sync.dma_start(out=gath_d[qh, 0:P, :], in_=att_d[qh])
                    else:
                        nc.gpsimd.collective_compute(
                            "AllGather", mybir.AluOpType.bypass,
                            replica_groups=RG,
                            ins=[att_d[qh].opt()],
                            outs=[gath_d[qh].opt()],
                        )

        # ---------------- Phase 3: o_proj (column slice) -----------------
        with tc.tile_pool(name="ph3_wo", bufs=1) as wop, \
             tc.tile_pool(name="ph3_g", bufs=2) as gp, \
             tc.tile_pool(name="ph3_o", bufs=3) as oop, \
             tc.tile_pool(name="ph3_ps", bufs=4, space="PSUM") as p3:
            wo_sb = wop.tile([P, AKT, OC], F32R, tag="wo")
            for kt in range(AKT):
                nc.sync.dma_start(out=wo_sb[:, kt, :], in_=wo_t[:, kt, :].bitcast(F32R))
            for sb in range(NSB):
                g_sb = gp.tile([P, QH, TP, P], F32R, tag="g")
                for h in range(QH):
                    if phase3_reads_att:
                        for r in range(TP):
                            nc.sync.dma_start(
                                out=g_sb[:, h, r, :],
                                in_=att_d[h][:, sb * P:(sb + 1) * P].bitcast(F32R),
                            )
                    else:
                        gd = gath_d[h].rearrange("(r p) s -> p r s", p=P)
                        nc.sync.dma_start(
                            out=g_sb[:, h, :, :], in_=gd[:, :, sb * P:(sb + 1) * P].bitcast(F32R)
                        )
                for oc in range(2):
                    ps = p3.tile([P, 512], F32, tag="o", name=f"o_{sb}_{oc}")
                    for h in range(QH):
                        for r in range(TP):
                            kt = 4 * h + r
                            nc.tensor.matmul(
                                ps[:],
                                g_sb[:, h, r, :],
                                wo_sb[:, kt, oc * 512:(oc + 1) * 512],
                                start=(kt == 0), stop=(kt == AKT - 1),
                            )
                    o_sb = oop.tile([P, 512], F32, tag="osb")
                    nc.scalar.copy(o_sb[:], ps[:])
                    nc.sync.dma_start(
                        out=out[sb * P:(sb + 1) * P, oc * 512:(oc + 1) * 512],
                        in_=o_sb[:],
                    )
    nc.compile()
    return nc


def _prep_inputs(hidden_states, w_qkv, w_o, positions):
    """Host-side sharding + weight permutation. Returns per-core in_maps."""
    hidden_states = np.asarray(hidden_states, dtype=np.float32)
    w_qkv = np.asarray(w_qkv, dtype=np.float32)
    w_o = np.asarray(w_o, dtype=np.float32)
    positions = np.asarray(positions)

    # neox permutation of q/k head columns (evens then odds within each head)
    perm = np.concatenate([np.arange(0, D, 2), np.arange(1, D, 2)])
    wq_all = w_qkv[:, :NQ * D].reshape(H, NQ, D)[:, :, perm]      # [H, NQ, D]
    wk_all = w_qkv[:, NQ * D:(NQ + NKV) * D].reshape(H, NKV, D)[:, :, perm]
    wv_all = w_qkv[:, (NQ + NKV) * D:].reshape(H, NKV, D)

    # o_proj row permutation to match chunked AllGather order:
    # k-tile (h, r) holds global head 8r + h.
    head_order = np.array([8 * r + h for h in range(QH) for r in range(TP)])
    wo_perm = w_o.reshape(NQ, D, H)[head_order]                   # [32, D, H]

    # cos/sin tables, duplicated halves; sin top rows negated.
    inv_freq = 1.0 / (THETA ** (np.arange(0, D, 2, dtype=np.float32) / D))
    in_maps = []
    for c in range(NCORES):
        b, t = c // TP, c % TP
        freqs = positions[b].astype(np.float32)[None, :] * inv_freq[:, None]
        cos = np.cos(freqs)                                       # [64, S]
        sin = np.sin(freqs)
        cosf = np.concatenate([cos, cos], axis=0).astype(np.float32)
        sins = np.concatenate([-sin, sin], axis=0).astype(np.float32)

        wq = wq_all[:, 8 * t:8 * t + 8].reshape(H, QC)
        wk = wk_all[:, 2 * t:2 * t + 2].reshape(H, KC)
        wv = wv_all[:, 2 * t:2 * t + 2].reshape(H, KC)
        in_maps.append({
            "hidT": np.ascontiguousarray(hidden_states[b].T),
            "wqk": np.ascontiguousarray(np.concatenate([wq, wk], axis=1)),
            "wv": np.ascontiguousarray(wv),
            "wo": np.ascontiguousarray(
                wo_perm[:, :, 1024 * t:1024 * (t + 1)].reshape(NQ * D, OC)
            ),
            "cosf": cosf,
            "sins": sins,
        })
    return in_maps


_NC_CACHE = {}


def kernel(hidden_states, w_qkv, w_o, positions, _trace=False):
    if "nc" not in _NC_CACHE:
        _NC_CACHE["nc"] = build_program()
    nc = _NC_CACHE["nc"]
    in_maps = _prep_inputs(hidden_states, w_qkv, w_o, positions)
    res = run_bass_kernel_spmd(nc, in_maps, list(range(NCORES)), trace=_trace)
    out_full = np.empty((B, S, H), dtype=np.float32)
    for c in range(NCORES):
        b, t = c // TP, c % TP
        out_full[b, :, 1024 * t:1024 * (t + 1)] = res.results[c]["out"]
    if _trace:
        kernel.last_exec_time_ns = res.exec_time_ns
        kernel.last_profile = res
    return out_full



# revision 2
# speedup vs baseline: 1.3986x; 1.3986x over previous
"""Trainium2 Bass kernel for CohereAttention (B=2, S=2048, H=4096, 32Q/8KV heads, D=128).

Sharding: 8 cores = 2 batch groups x 4 tensor-parallel (head) ranks.
Core c: batch b = c // 4, tp rank t = c % 4.
  - owns q-heads [8t, 8t+8), kv-heads [2t, 2t+2) (GQA-aligned), w_o col slice
    [1024t, 1024(t+1)).
  - attention outputs (transposed [d, s], 8 heads stacked on partitions) are
    AllGather'd across the 4 ranks of the batch group per 512-query group;
    o_proj consumes gathered groups while later groups' attention still runs.

All matmuls run in bf16 (inputs rounded host- or engine-side, fp32 PSUM
accumulation) — bf16 sustains the full 1 column/cycle PE rate where fp32r
costs ~1.35 cycles/col and plain fp32 4.

RoPE is restructured host-side: q/k weight columns are permuted to "neox"
halves (evens then odds) so the on-device rotation is two block copies plus
elementwise ops against duplicated cos/sin tables, fused into the PSUM
eviction of the projection.

Attention uses transposed scores sT[k, q] so exp'd scores feed PV matmuls
directly; the softmax denominator is a ones-vector matmul accumulated
alongside PV; normalization = approx-reciprocal + gpsimd partition broadcast
+ one vector multiply during PSUM eviction. Causal diagonal blocks are
trimmed to the live query range (N = 512-128r).
"""

import numpy as np

import concourse.bass as bass
import concourse.mybir as mybir
from concourse import bacc
import concourse.tile as tile
from concourse.bass_utils import run_bass_kernel_spmd

# Problem constants (fixed by the task).
B, S, H = 2, 2048, 4096
NQ, NKV, D = 32, 8, 128
THETA = 10000.0
NCORES = 8
TP = 4                      # head-parallel group size
QH = NQ // TP               # 8 q heads per core
KH = NKV // TP              # 2 kv heads per core
REP = NQ // NKV             # 4
SCALE = float(D) ** -0.5
QC = QH * D                 # 1024 local q cols
KC = KH * D                 # 256 local k cols
OC = H // TP                # 1024 output cols per core
P = 128
KT = H // P                 # 32 contraction tiles for projections
NSB = S // P                # 16 seq blocks
SG = 512
NSG = S // SG               # 4 seq groups
F32 = mybir.dt.float32
BF16 = mybir.dt.bfloat16
NCT = (QC + KC) // P        # 10 q/k col-tiles per core
RG = [[0, 1, 2, 3], [4, 5, 6, 7]]

Exp = mybir.ActivationFunctionType.Exp


def build_program():
    """Emit the SPMD Bass program (same program for all 8 cores)."""
    nc = bacc.Bacc('TRN2', target_bir_lowering=False, debug=False, num_devices=NCORES)

    # host-prepacked inputs (bf16 except trig tables / output)
    hid_r = nc.dram_tensor("hid_r", [2, P, KT, SG * 2], BF16, kind="ExternalInput")
    wqk_r = nc.dram_tensor("wqk_r", [NCT, P, KT, P], BF16, kind="ExternalInput")
    wv_r = nc.dram_tensor("wv_r", [P, KT, KC], BF16, kind="ExternalInput")
    wo_r = nc.dram_tensor("wo_r", [P, NQ, OC], BF16, kind="ExternalInput")
    cosf = nc.dram_tensor("cosf", [P, S], F32, kind="ExternalInput")
    sins = nc.dram_tensor("sins", [P, S], F32, kind="ExternalInput")
    out = nc.dram_tensor("out", [S, OC], F32, kind="ExternalOutput")

    att_d = nc.dram_tensor("att_d", [NSG, QH * P, SG], BF16)
    gath_d = nc.dram_tensor("gath_d", [NSG, TP * QH * P, SG], BF16)

    with tile.TileContext(nc) as tc:
        with tc.tile_pool(name="res", bufs=1) as res:
            # cross-phase residents
            qkT_sb = res.tile([P, NCT, 2, SG * 2], BF16, tag="qkT")   # 40 KB/part
            v_sb = res.tile([P, NSB, KC], BF16, tag="v")              # 8 KB/part
            ones_f = res.tile([P, 1], F32, tag="ones_f")
            nc.vector.memset(ones_f[:], 1.0)
            ones_bf = res.tile([P, 1], BF16, tag="ones")
            nc.vector.tensor_copy(ones_bf[:], ones_f[:])

            # ---------------- Phase 1: qkv projection + RoPE -------------
            with tc.tile_pool(name="p1_hid", bufs=1) as hidp, \
                 tc.tile_pool(name="p1_w", bufs=2) as wp, \
                 tc.tile_pool(name="p1_wv", bufs=1) as wvp, \
                 tc.tile_pool(name="p1_cs", bufs=1) as csp, \
                 tc.tile_pool(name="p1_rope", bufs=2) as rp, \
                 tc.tile_pool(name="p1_ps", bufs=2, space="PSUM") as pp, \
                 tc.tile_pool(name="p1_psv", bufs=2, space="PSUM") as ppv:
                cos_sb = csp.tile([P, S], F32, tag="cos")
                sin_sb = csp.tile([P, S], F32, tag="sin")
                nc.sync.dma_start(out=cos_sb[:], in_=cosf[:])
                nc.sync.dma_start(out=sin_sb[:], in_=sins[:])
                wv_sb = wvp.tile([P, KT, KC], BF16, tag="wv")
                for kk in range(4):
                    nc.sync.dma_start(
                        out=wv_sb[:, kk * 8:(kk + 1) * 8, :],
                        in_=wv_r[:, kk * 8:(kk + 1) * 8, :],
                    )
                for sg in range(2):          # seq halves of 1024
                    hid_sb = hidp.tile([P, KT, SG * 2], BF16, tag="hid")
                    for kk in range(8):
                        nc.sync.dma_start(
                            out=hid_sb[:, kk * 4:(kk + 1) * 4, :],
                            in_=hid_r[sg][:, kk * 4:(kk + 1) * 4, :],
                        )
                    for ct in range(NCT):
                        w_sb = wp.tile([P, KT, P], BF16, tag="w")
                        nc.sync.dma_start(out=w_sb[:], in_=wqk_r[ct])
                        ps0 = pp.tile([P, SG], F32, tag="ps0", name=f"ps0_{sg}_{ct}")
                        ps1 = pp.tile([P, SG], F32, tag="ps1", name=f"ps1_{sg}_{ct}")
                        for kt in range(KT):
                            nc.tensor.matmul(
                                ps0[:], w_sb[:, kt, :], hid_sb[:, kt, 0:SG],
                                start=(kt == 0), stop=(kt == KT - 1),
                            )
                            nc.tensor.matmul(
                                ps1[:], w_sb[:, kt, :], hid_sb[:, kt, SG:2 * SG],
                                start=(kt == 0), stop=(kt == KT - 1),
                            )
                        # fused RoPE eviction (neox halves; sin top rows negated)
                        css = slice(sg * 1024, sg * 1024 + 1024)
                        tmp = rp.tile([P, 2 * SG], F32, tag="tmp")
                        t1 = rp.tile([P, 2 * SG], F32, tag="t1")
                        nc.vector.tensor_copy(tmp[0:64, 0:SG], ps0[64:128, :])
                        nc.vector.tensor_copy(tmp[64:128, 0:SG], ps0[0:64, :])
                        nc.vector.tensor_copy(tmp[0:64, SG:2 * SG], ps1[64:128, :])
                        nc.vector.tensor_copy(tmp[64:128, SG:2 * SG], ps1[0:64, :])
                        nc.vector.tensor_mul(t1[:, 0:SG], ps0[:], cos_sb[:, sg * 1024:sg * 1024 + SG])
                        nc.vector.tensor_mul(t1[:, SG:2 * SG], ps1[:], cos_sb[:, sg * 1024 + SG:sg * 1024 + 2 * SG])
                        nc.vector.tensor_mul(tmp[:], tmp[:], sin_sb[:, css])
                        nc.vector.tensor_add(qkT_sb[:, ct, sg, :], t1[:], tmp[:])
                    # v projection: out[M=seq, N=vcols]
                    for sbl in range(8):
                        psv = ppv.tile([P, KC], F32, tag="psv", name=f"psv_{sg}_{sbl}")
                        for kt in range(KT):
                            nc.tensor.matmul(
                                psv[:],
                                hid_sb[:, kt, sbl * P:(sbl + 1) * P],
                                wv_sb[:, kt, :],
                                start=(kt == 0), stop=(kt == KT - 1),
                            )
                        nc.scalar.copy(v_sb[:, sg * 8 + sbl, :], psv[:])

            # ------------- Phase 2 + 3: attention, gather, o_proj --------
            with tc.tile_pool(name="p2_p", bufs=4) as ppl, \
                 tc.tile_pool(name="p2_o", bufs=2) as op, \
                 tc.tile_pool(name="p2_rb", bufs=2) as rbp, \
                 tc.tile_pool(name="p3_wo", bufs=1) as wop, \
                 tc.tile_pool(name="p3_g", bufs=2) as gp, \
                 tc.tile_pool(name="p3_o", bufs=2) as oop, \
                 tc.tile_pool(name="p2_ps", bufs=2, space="PSUM") as sp, \
                 tc.tile_pool(name="p2_pacc", bufs=2, space="PSUM") as ap_, \
                 tc.tile_pool(name="p2_pden", bufs=2, space="PSUM") as dp, \
                 tc.tile_pool(name="p3_ps", bufs=2, space="PSUM") as p3:
                wo_sb = wop.tile([P, NQ, OC], BF16, tag="wo")
                for kk in range(8):
                    nc.sync.dma_start(
                        out=wo_sb[:, kk * 4:(kk + 1) * 4, :],
                        in_=wo_r[:, kk * 4:(kk + 1) * 4, :],
                    )

                def qT(h, j, q0=0):
                    # query cols [j*SG + q0, (j+1)*SG) of head h
                    sg, off = (j * SG + q0) // 1024, (j * SG + q0) % 1024
                    return qkT_sb[:, h, sg, off:off + SG - q0]

                def kT(kv, c):
                    sg, off = (c * P) // 1024, (c * P) % 1024
                    return qkT_sb[:, QH + kv, sg, off:off + P]

                def p3_emit(j):
                    g_sb = gp.tile([P, NQ, SG], BF16, tag="g", name=f"g_{j}")
                    gd = gath_d[j].rearrange("(kt p) s -> p kt s", p=P)
                    for kk in range(4):
                        nc.sync.dma_start(
                            out=g_sb[:, kk * 8:(kk + 1) * 8, :],
                            in_=gd[:, kk * 8:(kk + 1) * 8, :],
                        )
                    for sb in range(4):
                        qs = slice(sb * P, (sb + 1) * P)
                        for ocl in range(2):
                            ps = p3.tile([P, SG], F32, tag="o", name=f"o_{j}_{sb}_{ocl}")
                            for kt in range(NQ):
                                nc.tensor.matmul(
                                    ps[:],
                                    g_sb[:, kt, qs],
                                    wo_sb[:, kt, ocl * SG:(ocl + 1) * SG],
                                    start=(kt == 0), stop=(kt == NQ - 1),
                                )
                            o_sb = oop.tile([P, SG], F32, tag="osb")
                            nc.vector.tensor_copy(o_sb[:], ps[:])
                            nc.sync.dma_start(
                                out=out[j * SG + sb * P:j * SG + (sb + 1) * P,
                                        ocl * SG:(ocl + 1) * SG],
                                in_=o_sb[:],
                            )

                for j in range(NSG):
                    ncb = 4 * j + 4
                    for h in range(QH):
                        kv = h // REP
                        att_ps = ap_.tile([P, SG], F32, tag="att", name=f"att_{j}_{h}")
                        den_ps = dp.tile([1, SG], F32, tag="den", name=f"den_{j}_{h}")
                        plist = []
                        for c in range(ncb):
                            r = c - 4 * j
                            q0 = max(r, 0) * P          # trimmed query offset
                            n = SG - q0
                            s_ps = sp.tile([P, SG], F32, tag="s", name=f"s_{j}_{h}_{c}")
                            nc.tensor.matmul(
                                s_ps[:, 0:n], kT(kv, c), qT(h, j, q0),
                                start=True, stop=True,
                            )
                            p_sb = ppl.tile([P, SG], BF16, tag="p", name=f"p_{j}_{h}_{c}")
                            nc.scalar.activation(p_sb[:, 0:n], s_ps[:, 0:n], Exp, scale=SCALE)
                            if r >= 0:
                                # zero p where key > query within diagonal chunk
                                nc.gpsimd.affine_select(
                                    out=p_sb[:, 0:n], in_=p_sb[:, 0:n],
                                    compare_op=mybir.AluOpType.is_ge,
                                    fill=0.0, base=0,
                                    pattern=[[1, n]], channel_multiplier=-1,
                                )
                            plist.append((c, q0, n, p_sb))
                            if len(plist) >= 3:
                                cc, cq0, cn, cp = plist.pop(0)
                                nc.tensor.matmul(
                                    den_ps[:, cq0:SG], ones_bf[:], cp[:, 0:cn],
                                    start=(cc == 0), stop=(cc == ncb - 1),
                                    skip_group_check=True,
                                )
                                nc.tensor.matmul(
                                    att_ps[:, cq0:SG], v_sb[:, cc, kv * P:(kv + 1) * P], cp[:, 0:cn],
                                    start=(cc == 0), stop=(cc == ncb - 1),
                                    skip_group_check=True,
                                )
                        for cc, cq0, cn, cp in plist:
                            nc.tensor.matmul(
                                den_ps[:, cq0:SG], ones_bf[:], cp[:, 0:cn],
                                start=(cc == 0), stop=(cc == ncb - 1),
                                skip_group_check=True,
                            )
                            nc.tensor.matmul(
                                att_ps[:, cq0:SG], v_sb[:, cc, kv * P:(kv + 1) * P], cp[:, 0:cn],
                                start=(cc == 0), stop=(cc == ncb - 1),
                                skip_group_check=True,
                            )
                        rinv = op.tile([1, SG], F32, tag="rinv")
                        nc.vector.reciprocal_approx_fast(rinv[:], den_ps[:])
                        rb_sb = rbp.tile([P, SG], F32, tag="rb", name=f"rb_{j}_{h}")
                        nc.gpsimd.partition_broadcast(rb_sb[:], rinv[:])
                        att_sb = op.tile([P, SG], BF16, tag="att_sb")
                        nc.vector.tensor_mul(att_sb[:], att_ps[:], rb_sb[:])
                        nc.sync.dma_start(
                            out=att_d[j, h * P:(h + 1) * P, :], in_=att_sb[:]
                        )
                    nc.gpsimd.collective_compute(
                        "AllGather", mybir.AluOpType.bypass,
                        replica_groups=RG,
                        ins=[att_d[j].opt()],
                        outs=[gath_d[j].opt()],
                    )
                    if j >= 1:
                        p3_emit(j - 1)
                p3_emit(NSG - 1)
    nc.compile()
    return nc


def _prep_inputs(hidden_states, w_qkv, w_o, positions):
    """Host-side sharding + bf16 packing. Returns per-core in_maps."""
    import ml_dtypes
    bf16 = ml_dtypes.bfloat16
    hidden_states = np.asarray(hidden_states, dtype=np.float32)
    w_qkv = np.asarray(w_qkv, dtype=np.float32)
    w_o = np.asarray(w_o, dtype=np.float32)
    positions = np.asarray(positions)

    # neox permutation of q/k head columns (evens then odds within each head)
    perm = np.concatenate([np.arange(0, D, 2), np.arange(1, D, 2)])
    wq_all = w_qkv[:, :NQ * D].reshape(H, NQ, D)[:, :, perm]      # [H, NQ, D]
    wk_all = w_qkv[:, NQ * D:(NQ + NKV) * D].reshape(H, NKV, D)[:, :, perm]
    wv_all = w_qkv[:, (NQ + NKV) * D:].reshape(H, NKV, D)

    inv_freq = 1.0 / (THETA ** (np.arange(0, D, 2, dtype=np.float32) / D))
    in_maps = []
    for c in range(NCORES):
        b, t = c // TP, c % TP
        freqs = positions[b].astype(np.float32)[None, :] * inv_freq[:, None]
        cos = np.cos(freqs)                                       # [64, S]
        sin = np.sin(freqs)
        cosf = np.concatenate([cos, cos], axis=0).astype(np.float32)
        sins = np.concatenate([-sin, sin], axis=0).astype(np.float32)

        # hid_r: [sg, p, kt, 1024] with h = kt*128 + p
        hid_r = np.ascontiguousarray(
            hidden_states[b].T.reshape(KT, P, 2, SG * 2).transpose(2, 1, 0, 3)
        ).astype(bf16)
        # wqk cols for this rank: 8 q heads then 2 kv heads (neox-permuted)
        wq = wq_all[:, 8 * t:8 * t + 8].reshape(H, QC)
        wk = wk_all[:, 2 * t:2 * t + 2].reshape(H, KC)
        wqk = np.concatenate([wq, wk], axis=1)                    # [H, 1280]
        # wqk_r: [ct, p, kt, 128] with h = kt*128 + p, col = ct*128 + i
        wqk_r = np.ascontiguousarray(
            wqk.reshape(KT, P, NCT, P).transpose(2, 1, 0, 3)
        ).astype(bf16)
        wv = wv_all[:, 2 * t:2 * t + 2].reshape(H, KC)
        wv_r = np.ascontiguousarray(
            wv.reshape(KT, P, KC).transpose(1, 0, 2)
        ).astype(bf16)
        # wo rows in natural global-head order (gather is rank-major:
        # kt = r*8 + h_local = global head); cols = this rank's slice.
        wo_r = np.ascontiguousarray(
            w_o.reshape(NQ, D, H)[:, :, OC * t:OC * (t + 1)].transpose(1, 0, 2)
        ).astype(bf16)
        in_maps.append({
            "hid_r": hid_r,
            "wqk_r": wqk_r,
            "wv_r": wv_r,
            "wo_r": wo_r,
            "cosf": cosf,
            "sins": sins,
        })
    return in_maps


_NC_CACHE = {}


def kernel(hidden_states, w_qkv, w_o, positions, _trace=False):
    if "nc" not in _NC_CACHE:
        _NC_CACHE["nc"] = build_program()
    nc = _NC_CACHE["nc"]
    in_maps = _prep_inputs(hidden_states, w_qkv, w_o, positions)
    res = run_bass_kernel_spmd(nc, in_maps, list(range(NCORES)), trace=_trace)
    out_full = np.empty((B, S, H), dtype=np.float32)
    for c in range(NCORES):
        b, t = c // TP, c % TP
        out_full[b, :, OC * t:OC * (t + 1)] = res.results[c]["out"]
    if _trace:
        kernel.last_exec_time_ns = res.exec_time_ns
        kernel.last_profile = res
    return out_full


# revision 6
# speedup vs baseline: 1.4385x; 1.0285x over previous
"""Trainium2 Bass kernel for CohereAttention (B=2, S=2048, H=4096, 32Q/8KV heads, D=128).

Sharding: 8 cores = 2 batch groups x 4 tensor-parallel (head) ranks.
Core c: batch b = c // 4, tp rank t = c % 4.
  - owns q-heads [8t, 8t+8), kv-heads [2t, 2t+2) (GQA-aligned), w_o col slice
    [1024t, 1024(t+1)).
  - attention outputs (transposed [d, s], 8 heads stacked on partitions) are
    AllGather'd across the 4 ranks of the batch group per 512-query group;
    o_proj consumes gathered groups while later groups' attention still runs.

All matmuls run in bf16 (inputs rounded host- or engine-side, fp32 PSUM
accumulation) — bf16 sustains the full 1 column/cycle PE rate where fp32r
costs ~1.35 cycles/col and plain fp32 4.

RoPE is restructured host-side: q/k weight columns are permuted to "neox"
halves (evens then odds) so the on-device rotation is two block copies plus
elementwise ops against duplicated cos/sin tables, fused into the PSUM
eviction of the projection.

Attention uses transposed scores sT[k, q] so exp'd scores feed PV matmuls
directly; the softmax denominator is a ones-vector matmul accumulated
alongside PV; normalization = approx-reciprocal + gpsimd partition broadcast
+ one vector multiply during PSUM eviction. Causal diagonal blocks are
trimmed to the live query range (N = 512-128r).
"""

import numpy as np

import concourse.bass as bass
import concourse.mybir as mybir
from concourse import bacc
import concourse.tile as tile
from concourse.bass_utils import run_bass_kernel_spmd

# Problem constants (fixed by the task).
B, S, H = 2, 2048, 4096
NQ, NKV, D = 32, 8, 128
THETA = 10000.0
NCORES = 8
TP = 4                      # head-parallel group size
QH = NQ // TP               # 8 q heads per core
KH = NKV // TP              # 2 kv heads per core
REP = NQ // NKV             # 4
SCALE = float(D) ** -0.5
QC = QH * D                 # 1024 local q cols
KC = KH * D                 # 256 local k cols
OC = H // TP                # 1024 output cols per core
P = 128
KT = H // P                 # 32 contraction tiles for projections
NSB = S // P                # 16 seq blocks
SG = 512
NSG = S // SG               # 4 seq groups
F32 = mybir.dt.float32
BF16 = mybir.dt.bfloat16
NCT = (QC + KC) // P        # 10 q/k col-tiles per core
RG = [[0, 1, 2, 3], [4, 5, 6, 7]]

Exp = mybir.ActivationFunctionType.Exp


def build_program():
    """Emit the SPMD Bass program (same program for all 8 cores)."""
    nc = bacc.Bacc('TRN2', target_bir_lowering=False, debug=False, num_devices=NCORES)

    # host-prepacked inputs (bf16 except trig tables / output)
    hid_r = nc.dram_tensor("hid_r", [2, P, KT, SG * 2], BF16, kind="ExternalInput")
    wqk_r = nc.dram_tensor("wqk_r", [NCT, P, KT, P], BF16, kind="ExternalInput")
    wv_r = nc.dram_tensor("wv_r", [P, KT, KC], BF16, kind="ExternalInput")
    wo_r = nc.dram_tensor("wo_r", [P, NQ, OC], BF16, kind="ExternalInput")
    cosf = nc.dram_tensor("cosf", [P, S], F32, kind="ExternalInput")
    sins = nc.dram_tensor("sins", [P, S], F32, kind="ExternalInput")
    out = nc.dram_tensor("out", [S, OC], F32, kind="ExternalOutput")

    # per half-group (4 heads) gather buffers: earlier, smaller collectives
    att_d = nc.dram_tensor("att_d", [NSG * 2, QH // 2 * P, SG], BF16)
    gath_d = nc.dram_tensor("gath_d", [NSG * 2, TP * QH // 2 * P, SG], BF16)

    with tile.TileContext(nc) as tc:
        with tc.tile_pool(name="res", bufs=1) as res:
            # cross-phase residents
            qkT_sb = res.tile([P, NCT, 2, SG * 2], BF16, tag="qkT")   # 40 KB/part
            v_sb = res.tile([P, NSB, KC], BF16, tag="v")              # 8 KB/part
            ones_f = res.tile([P, 1], F32, tag="ones_f")
            nc.vector.memset(ones_f[:], 1.0)
            ones_bf = res.tile([P, 1], BF16, tag="ones")
            nc.vector.tensor_copy(ones_bf[:], ones_f[:])

            # ---------------- Phase 1: qkv projection + RoPE -------------
            with tc.tile_pool(name="p1_hid", bufs=1) as hidp, \
                 tc.tile_pool(name="p1_w", bufs=2) as wp, \
                 tc.tile_pool(name="p1_wv", bufs=1) as wvp, \
                 tc.tile_pool(name="p1_cs", bufs=1) as csp, \
                 tc.tile_pool(name="p1_rope", bufs=2) as rp, \
                 tc.tile_pool(name="p1_ps", bufs=2, space="PSUM") as pp, \
                 tc.tile_pool(name="p1_psv", bufs=2, space="PSUM") as ppv:
                cos_sb = csp.tile([P, S], F32, tag="cos")
                sin_sb = csp.tile([P, S], F32, tag="sin")
                nc.sync.dma_start(out=cos_sb[:], in_=cosf[:])
                nc.sync.dma_start(out=sin_sb[:], in_=sins[:])
                wv_sb = wvp.tile([P, KT, KC], BF16, tag="wv")
                for kk in range(4):
                    nc.sync.dma_start(
                        out=wv_sb[:, kk * 8:(kk + 1) * 8, :],
                        in_=wv_r[:, kk * 8:(kk + 1) * 8, :],
                    )
                for sg in range(2):          # seq halves of 1024
                    hid_sb = hidp.tile([P, KT, SG * 2], BF16, tag="hid")
                    for kk in range(8):
                        nc.sync.dma_start(
                            out=hid_sb[:, kk * 4:(kk + 1) * 4, :],
                            in_=hid_r[sg][:, kk * 4:(kk + 1) * 4, :],
                        )
                    for ct in range(NCT):
                        w_sb = wp.tile([P, KT, P], BF16, tag="w")
                        nc.sync.dma_start(out=w_sb[:], in_=wqk_r[ct])
                        ps0 = pp.tile([P, SG], F32, tag="ps0", name=f"ps0_{sg}_{ct}")
                        ps1 = pp.tile([P, SG], F32, tag="ps1", name=f"ps1_{sg}_{ct}")
                        for kt in range(KT):
                            nc.tensor.matmul(
                                ps0[:], w_sb[:, kt, :], hid_sb[:, kt, 0:SG],
                                start=(kt == 0), stop=(kt == KT - 1),
                            )
                            nc.tensor.matmul(
                                ps1[:], w_sb[:, kt, :], hid_sb[:, kt, SG:2 * SG],
                                start=(kt == 0), stop=(kt == KT - 1),
                            )
                        # fused RoPE eviction (neox halves; sin top rows negated)
                        css = slice(sg * 1024, sg * 1024 + 1024)
                        tmp = rp.tile([P, 2 * SG], F32, tag="tmp")
                        t1 = rp.tile([P, 2 * SG], F32, tag="t1")
                        nc.vector.tensor_copy(tmp[0:64, 0:SG], ps0[64:128, :])
                        nc.vector.tensor_copy(tmp[64:128, 0:SG], ps0[0:64, :])
                        nc.vector.tensor_copy(tmp[0:64, SG:2 * SG], ps1[64:128, :])
                        nc.vector.tensor_copy(tmp[64:128, SG:2 * SG], ps1[0:64, :])
                        nc.vector.tensor_mul(t1[:, 0:SG], ps0[:], cos_sb[:, sg * 1024:sg * 1024 + SG])
                        nc.vector.tensor_mul(t1[:, SG:2 * SG], ps1[:], cos_sb[:, sg * 1024 + SG:sg * 1024 + 2 * SG])
                        nc.vector.tensor_mul(tmp[:], tmp[:], sin_sb[:, css])
                        nc.vector.tensor_add(qkT_sb[:, ct, sg, :], t1[:], tmp[:])
                    # v projection: out[M=seq, N=vcols]
                    for sbl in range(8):
                        psv = ppv.tile([P, KC], F32, tag="psv", name=f"psv_{sg}_{sbl}")
                        for kt in range(KT):
                            nc.tensor.matmul(
                                psv[:],
                                hid_sb[:, kt, sbl * P:(sbl + 1) * P],
                                wv_sb[:, kt, :],
                                start=(kt == 0), stop=(kt == KT - 1),
                            )
                        nc.scalar.copy(v_sb[:, sg * 8 + sbl, :], psv[:])

            # ------------- Phase 2 + 3: attention, gather, o_proj --------
            with tc.tile_pool(name="p2_p", bufs=4) as ppl, \
                 tc.tile_pool(name="p2_o", bufs=2) as op, \
                 tc.tile_pool(name="p2_rb", bufs=2) as rbp, \
                 tc.tile_pool(name="p3_wo", bufs=1) as wop, \
                 tc.tile_pool(name="p3_g", bufs=2) as gp, \
                 tc.tile_pool(name="p3_o", bufs=2) as oop, \
                 tc.tile_pool(name="p2_ps", bufs=2, space="PSUM") as sp, \
                 tc.tile_pool(name="p2_pacc", bufs=2, space="PSUM") as ap_, \
                 tc.tile_pool(name="p2_pden", bufs=2, space="PSUM") as dp, \
                 tc.tile_pool(name="p3_ps", bufs=2, space="PSUM") as p3:
                wo_sb = wop.tile([P, NQ, OC], BF16, tag="wo")
                for kk in range(8):
                    nc.sync.dma_start(
                        out=wo_sb[:, kk * 4:(kk + 1) * 4, :],
                        in_=wo_r[:, kk * 4:(kk + 1) * 4, :],
                    )

                def qT(h, j, q0=0):
                    # query cols [j*SG + q0, (j+1)*SG) of head h
                    sg, off = (j * SG + q0) // 1024, (j * SG + q0) % 1024
                    return qkT_sb[:, h, sg, off:off + SG - q0]

                def kT(kv, c):
                    sg, off = (c * P) // 1024, (c * P) % 1024
                    return qkT_sb[:, QH + kv, sg, off:off + P]

                def p3_emit(j):
                    g_sb = gp.tile([P, NQ, SG], BF16, tag="g", name=f"g_{j}")
                    for half in range(2):
                        gd = gath_d[2 * j + half].rearrange("(kt p) s -> p kt s", p=P)
                        for kk in range(2):
                            nc.sync.dma_start(
                                out=g_sb[:, half * 16 + kk * 8:half * 16 + (kk + 1) * 8, :],
                                in_=gd[:, kk * 8:(kk + 1) * 8, :],
                            )
                    for sb in range(4):
                        qs = slice(sb * P, (sb + 1) * P)
                        for ocl in range(2):
                            ps = p3.tile([P, SG], F32, tag="o", name=f"o_{j}_{sb}_{ocl}")
                            for kt in range(NQ):
                                nc.tensor.matmul(
                                    ps[:],
                                    g_sb[:, kt, qs],
                                    wo_sb[:, kt, ocl * SG:(ocl + 1) * SG],
                                    start=(kt == 0), stop=(kt == NQ - 1),
                                )
                            o_sb = oop.tile([P, SG], F32, tag="osb")
                            nc.vector.tensor_copy(o_sb[:], ps[:])
                            nc.sync.dma_start(
                                out=out[j * SG + sb * P:j * SG + (sb + 1) * P,
                                        ocl * SG:(ocl + 1) * SG],
                                in_=o_sb[:],
                            )

                for j in range(NSG):
                    ncb = 4 * j + 4
                    for h in range(QH):
                        kv = h // REP
                        att_ps = ap_.tile([P, SG], F32, tag="att", name=f"att_{j}_{h}")
                        den_ps = dp.tile([1, SG], F32, tag="den", name=f"den_{j}_{h}")
                        plist = []
                        for c in range(ncb):
                            r = c - 4 * j
                            q0 = max(r, 0) * P          # trimmed query offset
                            n = SG - q0
                            s_ps = sp.tile([P, SG], F32, tag="s", name=f"s_{j}_{h}_{c}")
                            nc.tensor.matmul(
                                s_ps[:, 0:n], kT(kv, c), qT(h, j, q0),
                                start=True, stop=True,
                            )
                            p_sb = ppl.tile([P, SG], BF16, tag="p", name=f"p_{j}_{h}_{c}")
                            nc.scalar.activation(p_sb[:, 0:n], s_ps[:, 0:n], Exp, scale=SCALE)
                            if r >= 0:
                                # zero p where key > query within diagonal chunk
                                nc.gpsimd.affine_select(
                                    out=p_sb[:, 0:n], in_=p_sb[:, 0:n],
                                    compare_op=mybir.AluOpType.is_ge,
                                    fill=0.0, base=0,
                                    pattern=[[1, n]], channel_multiplier=-1,
                                )
                            plist.append((c, q0, n, p_sb))
                            if len(plist) >= 3:
                                cc, cq0, cn, cp = plist.pop(0)
                                nc.tensor.matmul(
                                    den_ps[:, cq0:SG], ones_bf[:], cp[:, 0:cn],
                                    start=(cc == 0), stop=(cc == ncb - 1),
                                    skip_group_check=True,
                                )
                                nc.tensor.matmul(
                                    att_ps[:, cq0:SG], v_sb[:, cc, kv * P:(kv + 1) * P], cp[:, 0:cn],
                                    start=(cc == 0), stop=(cc == ncb - 1),
                                    skip_group_check=True,
                                )
                        for cc, cq0, cn, cp in plist:
                            nc.tensor.matmul(
                                den_ps[:, cq0:SG], ones_bf[:], cp[:, 0:cn],
                                start=(cc == 0), stop=(cc == ncb - 1),
                                skip_group_check=True,
                            )
                            nc.tensor.matmul(
                                att_ps[:, cq0:SG], v_sb[:, cc, kv * P:(kv + 1) * P], cp[:, 0:cn],
                                start=(cc == 0), stop=(cc == ncb - 1),
                                skip_group_check=True,
                            )
                        rinv = op.tile([1, SG], F32, tag="rinv")
                        nc.vector.reciprocal_approx_fast(rinv[:], den_ps[:])
                        rb_sb = rbp.tile([P, SG], F32, tag="rb", name=f"rb_{j}_{h}")
                        nc.gpsimd.partition_broadcast(rb_sb[:], rinv[:])
                        att_sb = op.tile([P, SG], BF16, tag="att_sb")
                        nc.vector.tensor_mul(att_sb[:], att_ps[:], rb_sb[:])
                        nc.sync.dma_start(
                            out=att_d[2 * j + h // 4, (h % 4) * P:(h % 4 + 1) * P, :],
                            in_=att_sb[:],
                        )
                        if h % 4 == 3:
                            # gather this half-group as soon as its 4 heads land
                            nc.gpsimd.collective_compute(
                                "AllGather", mybir.AluOpType.bypass,
                                replica_groups=RG,
                                ins=[att_d[2 * j + h // 4].opt()],
                                outs=[gath_d[2 * j + h // 4].opt()],
                            )
                    if j >= 2:
                        p3_emit(j - 2)
                p3_emit(NSG - 2)
                p3_emit(NSG - 1)
    nc.compile()
    return nc


def _prep_inputs(hidden_states, w_qkv, w_o, positions):
    """Host-side sharding + bf16 packing. Returns per-core in_maps."""
    import ml_dtypes
    bf16 = ml_dtypes.bfloat16
    hidden_states = np.asarray(hidden_states, dtype=np.float32)
    w_qkv = np.asarray(w_qkv, dtype=np.float32)
    w_o = np.asarray(w_o, dtype=np.float32)
    positions = np.asarray(positions)

    # neox permutation of q/k head columns (evens then odds within each head)
    perm = np.concatenate([np.arange(0, D, 2), np.arange(1, D, 2)])
    wq_all = w_qkv[:, :NQ * D].reshape(H, NQ, D)[:, :, perm]      # [H, NQ, D]
    wk_all = w_qkv[:, NQ * D:(NQ + NKV) * D].reshape(H, NKV, D)[:, :, perm]
    wv_all = w_qkv[:, (NQ + NKV) * D:].reshape(H, NKV, D)

    inv_freq = 1.0 / (THETA ** (np.arange(0, D, 2, dtype=np.float32) / D))
    in_maps = []
    for c in range(NCORES):
        b, t = c // TP, c % TP
        freqs = positions[b].astype(np.float32)[None, :] * inv_freq[:, None]
        cos = np.cos(freqs)                                       # [64, S]
        sin = np.sin(freqs)
        cosf = np.concatenate([cos, cos], axis=0).astype(np.float32)
        sins = np.concatenate([-sin, sin], axis=0).astype(np.float32)

        # hid_r: [sg, p, kt, 1024] with h = kt*128 + p
        hid_r = np.ascontiguousarray(
            hidden_states[b].T.reshape(KT, P, 2, SG * 2).transpose(2, 1, 0, 3)
        ).astype(bf16)
        # wqk cols for this rank: 8 q heads then 2 kv heads (neox-permuted)
        wq = wq_all[:, 8 * t:8 * t + 8].reshape(H, QC)
        wk = wk_all[:, 2 * t:2 * t + 2].reshape(H, KC)
        wqk = np.concatenate([wq, wk], axis=1)                    # [H, 1280]
        # wqk_r: [ct, p, kt, 128] with h = kt*128 + p, col = ct*128 + i
        wqk_r = np.ascontiguousarray(
            wqk.reshape(KT, P, NCT, P).transpose(2, 1, 0, 3)
        ).astype(bf16)
        wv = wv_all[:, 2 * t:2 * t + 2].reshape(H, KC)
        wv_r = np.ascontiguousarray(
            wv.reshape(KT, P, KC).transpose(1, 0, 2)
        ).astype(bf16)
        # wo rows ordered to match half-group gathers: o_proj k-tile
        # kt = half*16 + r*4 + h2 holds global head 8r + 4*half + h2.
        head_order = np.array(
            [8 * r + 4 * half + h2
             for half in range(2) for r in range(TP) for h2 in range(4)]
        )
        wo_r = np.ascontiguousarray(
            w_o.reshape(NQ, D, H)[head_order, :, OC * t:OC * (t + 1)]
            .transpose(1, 0, 2)
        ).astype(bf16)
        in_maps.append({
            "hid_r": hid_r,
            "wqk_r": wqk_r,
            "wv_r": wv_r,
            "wo_r": wo_r,
            "cosf": cosf,
            "sins": sins,
        })
    return in_maps


_NC_CACHE = {}


def kernel(hidden_states, w_qkv, w_o, positions, _trace=False):
    if "nc" not in _NC_CACHE:
        _NC_CACHE["nc"] = build_program()
    nc = _NC_CACHE["nc"]
    in_maps = _prep_inputs(hidden_states, w_qkv, w_o, positions)
    res = run_bass_kernel_spmd(nc, in_maps, list(range(NCORES)), trace=_trace)
    out_full = np.empty((B, S, H), dtype=np.float32)
    for c in range(NCORES):
        b, t = c // TP, c % TP
        out_full[b, :, OC * t:OC * (t + 1)] = res.results[c]["out"]
    if _trace:
        kernel.last_exec_time_ns = res.exec_time_ns
        kernel.last_profile = res
    return out_full


# revision 15
# speedup vs baseline: 1.5067x; 1.0474x over previous
"""Trainium2 Bass kernel for CohereAttention (B=2, S=2048, H=4096, 32Q/8KV heads, D=128).

Sharding: 8 cores = 2 batch groups x 4 tensor-parallel (head) ranks.
Core c: batch b = c // 4, tp rank t = c % 4.
  - owns q-heads [8t, 8t+8), kv-heads [2t, 2t+2) (GQA-aligned), w_o col slice
    [1024t, 1024(t+1)).
  - attention outputs (transposed [d, s], 8 heads stacked on partitions) are
    AllGather'd across the 4 ranks of the batch group per 512-query group;
    o_proj consumes gathered groups while later groups' attention still runs.

All matmuls run in bf16 (inputs rounded host- or engine-side, fp32 PSUM
accumulation) — bf16 sustains the full 1 column/cycle PE rate where fp32r
costs ~1.35 cycles/col and plain fp32 4.

RoPE is restructured host-side: q/k weight columns are permuted to "neox"
halves (evens then odds) so the on-device rotation is two block copies plus
elementwise ops against duplicated cos/sin tables, fused into the PSUM
eviction of the projection.

Attention uses transposed scores sT[k, q] so exp'd scores feed PV matmuls
directly; the softmax denominator is a ones-vector matmul accumulated
alongside PV; normalization = approx-reciprocal + gpsimd partition broadcast
+ one vector multiply during PSUM eviction. Causal diagonal blocks are
trimmed to the live query range (N = 512-128r).
"""

import numpy as np

import concourse.bass as bass
import concourse.mybir as mybir
from concourse import bacc
import concourse.tile as tile
from concourse.bass_utils import run_bass_kernel_spmd

# Problem constants (fixed by the task).
B, S, H = 2, 2048, 4096
NQ, NKV, D = 32, 8, 128
THETA = 10000.0
NCORES = 8
TP = 4                      # head-parallel group size
QH = NQ // TP               # 8 q heads per core
KH = NKV // TP              # 2 kv heads per core
REP = NQ // NKV             # 4
SCALE = float(D) ** -0.5
QC = QH * D                 # 1024 local q cols
KC = KH * D                 # 256 local k cols
OC = H // TP                # 1024 output cols per core
P = 128
KT = H // P                 # 32 contraction tiles for projections
NSB = S // P                # 16 seq blocks
SG = 512
NSG = S // SG               # 4 seq groups
F32 = mybir.dt.float32
BF16 = mybir.dt.bfloat16
NCT = (QC + KC) // P        # 10 q/k col-tiles per core
RG = [[0, 1, 2, 3], [4, 5, 6, 7]]

Exp = mybir.ActivationFunctionType.Exp


def build_program():
    """Emit the SPMD Bass program (same program for all 8 cores)."""
    nc = bacc.Bacc('TRN2', target_bir_lowering=False, debug=False, num_devices=NCORES)

    # host-prepacked inputs (bf16 except trig tables / output)
    hid_r = nc.dram_tensor("hid_r", [NSG, P, KT, SG], BF16, kind="ExternalInput")
    wqk_r = nc.dram_tensor("wqk_r", [NCT, P, KT, P], BF16, kind="ExternalInput")
    wv_r = nc.dram_tensor("wv_r", [P, KT, KC], BF16, kind="ExternalInput")
    wo_r = nc.dram_tensor("wo_r", [P, NQ, OC], BF16, kind="ExternalInput")
    cosf = nc.dram_tensor("cosf", [P, S], F32, kind="ExternalInput")
    sins = nc.dram_tensor("sins", [P, S], F32, kind="ExternalInput")
    out = nc.dram_tensor("out", [S, OC], F32, kind="ExternalOutput")

    # per half-group (4 heads) gather buffers: earlier, smaller collectives
    att_d = nc.dram_tensor("att_d", [NSG * 2, QH // 2 * P, SG], BF16)
    gath_d = nc.dram_tensor("gath_d", [NSG * 2, TP * QH // 2 * P, SG], BF16)

    with tile.TileContext(nc) as tc:
        with tc.tile_pool(name="res", bufs=1) as res:
            # cross-phase residents
            qkT_sb = res.tile([P, NCT, S], BF16, tag="qkT")           # 40 KB/part
            v_sb = res.tile([P, NSB, KC], BF16, tag="v")              # 8 KB/part
            ones_f = res.tile([P, 1], F32, tag="ones_f")
            nc.vector.memset(ones_f[:], 1.0)
            ones_bf = res.tile([P, 1], BF16, tag="ones")
            nc.vector.tensor_copy(ones_bf[:], ones_f[:])
            # static causal masks for the 4 trimmed diagonal-chunk shapes:
            # mask_r[k, qq] = (qq >= k), width 512-128r. Applied by a vector
            # multiply so the gpsimd queue (collectives) is never in the
            # score->exp->PV critical path.
            masks = []
            for r in range(4):
                n = SG - r * P
                mk = res.tile([P, n], BF16, tag=f"mask{r}", name=f"mask{r}")
                nc.vector.memset(mk[:], 1.0)
                nc.gpsimd.affine_select(
                    out=mk[:], in_=mk[:],
                    compare_op=mybir.AluOpType.is_ge,
                    fill=0.0, base=0,
                    pattern=[[1, n]], channel_multiplier=-1,
                )
                masks.append(mk)

            # ---------------- Phase 1: qkv projection + RoPE -------------
            # seq processed in 4 quarters of 512; hid double-buffered so the
            # next quarter's activations stream in behind the current compute.
            with tc.tile_pool(name="p1_hid", bufs=2) as hidp, \
                 tc.tile_pool(name="p1_w", bufs=2) as wp, \
                 tc.tile_pool(name="p1_wv", bufs=1) as wvp, \
                 tc.tile_pool(name="p1_cs", bufs=1) as csp, \
                 tc.tile_pool(name="p1_rope", bufs=2) as rp, \
                 tc.tile_pool(name="p1_ps", bufs=2, space="PSUM") as pp, \
                 tc.tile_pool(name="p1_psv", bufs=2, space="PSUM") as ppv:
                cos_sb = csp.tile([P, S], F32, tag="cos")
                sin_sb = csp.tile([P, S], F32, tag="sin")
                nc.sync.dma_start(out=cos_sb[:], in_=cosf[:])
                nc.sync.dma_start(out=sin_sb[:], in_=sins[:])
                wv_sb = wvp.tile([P, KT, KC], BF16, tag="wv")
                for kk in range(4):
                    nc.sync.dma_start(
                        out=wv_sb[:, kk * 8:(kk + 1) * 8, :],
                        in_=wv_r[:, kk * 8:(kk + 1) * 8, :],
                    )
                for q in range(NSG):         # seq quarters of 512
                    qsl = slice(q * SG, (q + 1) * SG)
                    hid_sb = hidp.tile([P, KT, SG], BF16, tag="hid", name=f"hid_{q}")
                    for kk in range(4):
                        nc.sync.dma_start(
                            out=hid_sb[:, kk * 8:(kk + 1) * 8, :],
                            in_=hid_r[q][:, kk * 8:(kk + 1) * 8, :],
                        )
                    for ct in range(NCT):
                        w_sb = wp.tile([P, KT, P], BF16, tag="w", name=f"w_{q}_{ct}")
                        nc.sync.dma_start(out=w_sb[:], in_=wqk_r[ct])
                        ps = pp.tile([P, SG], F32, tag="ps", name=f"ps_{q}_{ct}")
                        for kt in range(KT):
                            nc.tensor.matmul(
                                ps[:], w_sb[:, kt, :], hid_sb[:, kt, :],
                                start=(kt == 0), stop=(kt == KT - 1),
                            )
                        # fused RoPE eviction (neox halves; sin top rows negated)
                        tmp = rp.tile([P, SG], F32, tag="tmp")
                        t1 = rp.tile([P, SG], F32, tag="t1")
                        nc.vector.tensor_copy(tmp[0:64, :], ps[64:128, :])
                        nc.vector.tensor_copy(tmp[64:128, :], ps[0:64, :])
                        nc.vector.tensor_mul(t1[:], ps[:], cos_sb[:, qsl])
                        nc.vector.tensor_mul(tmp[:], tmp[:], sin_sb[:, qsl])
                        nc.vector.tensor_add(qkT_sb[:, ct, qsl], t1[:], tmp[:])
                    # v projection: out[M=seq, N=vcols]
                    for sbl in range(4):
                        psv = ppv.tile([P, KC], F32, tag="psv", name=f"psv_{q}_{sbl}")
                        for kt in range(KT):
                            nc.tensor.matmul(
                                psv[:],
                                hid_sb[:, kt, sbl * P:(sbl + 1) * P],
                                wv_sb[:, kt, :],
                                start=(kt == 0), stop=(kt == KT - 1),
                            )
                        nc.scalar.copy(v_sb[:, q * 4 + sbl, :], psv[:])

            # ------------- Phase 2 + 3: attention, gather, o_proj --------
            with tc.tile_pool(name="p2_p", bufs=4) as ppl, \
                 tc.tile_pool(name="p2_o", bufs=2) as op, \
                 tc.tile_pool(name="p2_rb", bufs=2) as rbp, \
                 tc.tile_pool(name="p3_wo", bufs=1) as wop, \
                 tc.tile_pool(name="p3_g", bufs=2) as gp, \
                 tc.tile_pool(name="p3_o", bufs=2) as oop, \
                 tc.tile_pool(name="p2_ps", bufs=2, space="PSUM") as sp, \
                 tc.tile_pool(name="p2_pacc", bufs=2, space="PSUM") as ap_, \
                 tc.tile_pool(name="p2_pden", bufs=2, space="PSUM") as dp, \
                 tc.tile_pool(name="p3_ps", bufs=2, space="PSUM") as p3:
                wo_sb = wop.tile([P, NQ, OC], BF16, tag="wo")
                for kk in range(8):
                    nc.sync.dma_start(
                        out=wo_sb[:, kk * 4:(kk + 1) * 4, :],
                        in_=wo_r[:, kk * 4:(kk + 1) * 4, :],
                    )

                def qT(h, j, q0=0):
                    # query cols [j*SG + q0, (j+1)*SG) of head h
                    return qkT_sb[:, h, j * SG + q0:(j + 1) * SG]

                def kT(kv, c):
                    return qkT_sb[:, QH + kv, c * P:(c + 1) * P]

                def p3_emit(j):
                    g_sb = gp.tile([P, NQ, SG], BF16, tag="g", name=f"g_{j}")
                    for half in range(2):
                        gd = gath_d[2 * j + half].rearrange("(kt p) s -> p kt s", p=P)
                        for kk in range(2):
                            nc.sync.dma_start(
                                out=g_sb[:, half * 16 + kk * 8:half * 16 + (kk + 1) * 8, :],
                                in_=gd[:, kk * 8:(kk + 1) * 8, :],
                            )
                    for sb in range(4):
                        qs = slice(sb * P, (sb + 1) * P)
                        for ocl in range(2):
                            ps = p3.tile([P, SG], F32, tag="o", name=f"o_{j}_{sb}_{ocl}")
                            for kt in range(NQ):
                                nc.tensor.matmul(
                                    ps[:],
                                    g_sb[:, kt, qs],
                                    wo_sb[:, kt, ocl * SG:(ocl + 1) * SG],
                                    start=(kt == 0), stop=(kt == NQ - 1),
                                )
                            o_sb = oop.tile([P, SG], F32, tag="osb")
                            nc.vector.tensor_copy(o_sb[:], ps[:])
                            nc.sync.dma_start(
                                out=out[j * SG + sb * P:j * SG + (sb + 1) * P,
                                        ocl * SG:(ocl + 1) * SG],
                                in_=o_sb[:],
                            )

                for j in range(NSG):
                    ncb = 4 * j + 4
                    for h in range(QH):
                        kv = h // REP
                        att_ps = ap_.tile([P, SG], F32, tag="att", name=f"att_{j}_{h}")
                        den_ps = dp.tile([1, SG], F32, tag="den", name=f"den_{j}_{h}")
                        plist = []
                        for c in range(ncb):
                            r = c - 4 * j
                            q0 = max(r, 0) * P          # trimmed query offset
                            n = SG - q0
                            s_ps = sp.tile([P, SG], F32, tag="s", name=f"s_{j}_{h}_{c}")
                            nc.tensor.matmul(
                                s_ps[:, 0:n], kT(kv, c), qT(h, j, q0),
                                start=True, stop=True,
                            )
                            p_sb = ppl.tile([P, SG], BF16, tag="p", name=f"p_{j}_{h}_{c}")
                            nc.scalar.activation(p_sb[:, 0:n], s_ps[:, 0:n], Exp, scale=SCALE)
                            if r >= 0:
                                # zero p where key > query within diagonal chunk
                                nc.vector.tensor_mul(
                                    p_sb[:, 0:n], p_sb[:, 0:n], masks[r][:]
                                )
                            plist.append((c, q0, n, p_sb))
                            if len(plist) >= 3:
                                cc, cq0, cn, cp = plist.pop(0)
                                nc.tensor.matmul(
                                    den_ps[:, cq0:SG], ones_bf[:], cp[:, 0:cn],
                                    start=(cc == 0), stop=(cc == ncb - 1),
                                    skip_group_check=True,
                                )
                                nc.tensor.matmul(
                                    att_ps[:, cq0:SG], v_sb[:, cc, kv * P:(kv + 1) * P], cp[:, 0:cn],
                                    start=(cc == 0), stop=(cc == ncb - 1),
                                    skip_group_check=True,
                                )
                        for cc, cq0, cn, cp in plist:
                            nc.tensor.matmul(
                                den_ps[:, cq0:SG], ones_bf[:], cp[:, 0:cn],
                                start=(cc == 0), stop=(cc == ncb - 1),
                                skip_group_check=True,
                            )
                            nc.tensor.matmul(
                                att_ps[:, cq0:SG], v_sb[:, cc, kv * P:(kv + 1) * P], cp[:, 0:cn],
                                start=(cc == 0), stop=(cc == ncb - 1),
                                skip_group_check=True,
                            )
                        rinv = op.tile([1, SG], F32, tag="rinv")
                        nc.vector.reciprocal_approx_fast(rinv[:], den_ps[:])
                        rb_sb = rbp.tile([P, SG], F32, tag="rb", name=f"rb_{j}_{h}")
                        nc.gpsimd.partition_broadcast(rb_sb[:], rinv[:])
                        att_sb = op.tile([P, SG], BF16, tag="att_sb")
                        nc.vector.tensor_mul(att_sb[:], att_ps[:], rb_sb[:])
                        nc.sync.dma_start(
                            out=att_d[2 * j + h // 4, (h % 4) * P:(h % 4 + 1) * P, :],
                            in_=att_sb[:],
                        )
                        if h % 4 == 3:
                            # gather this half-group as soon as its 4 heads
                            nc.gpsimd.collective_compute(
                                "AllGather", mybir.AluOpType.bypass,
                                replica_groups=RG,
                                ins=[att_d[2 * j + h // 4].opt()],
                                outs=[gath_d[2 * j + h // 4].opt()],
                            )
                    if j >= 2:
                        p3_emit(j - 2)
                p3_emit(NSG - 2)
                p3_emit(NSG - 1)
    nc.compile()
    return nc


def _prep_inputs(hidden_states, w_qkv, w_o, positions):
    """Host-side sharding + bf16 packing. Returns per-core in_maps."""
    import ml_dtypes
    bf16 = ml_dtypes.bfloat16
    hidden_states = np.asarray(hidden_states, dtype=np.float32)
    w_qkv = np.asarray(w_qkv, dtype=np.float32)
    w_o = np.asarray(w_o, dtype=np.float32)
    positions = np.asarray(positions)

    # neox permutation of q/k head columns (evens then odds within each head)
    perm = np.concatenate([np.arange(0, D, 2), np.arange(1, D, 2)])
    wq_all = w_qkv[:, :NQ * D].reshape(H, NQ, D)[:, :, perm]      # [H, NQ, D]
    wk_all = w_qkv[:, NQ * D:(NQ + NKV) * D].reshape(H, NKV, D)[:, :, perm]
    wv_all = w_qkv[:, (NQ + NKV) * D:].reshape(H, NKV, D)

    inv_freq = 1.0 / (THETA ** (np.arange(0, D, 2, dtype=np.float32) / D))
    in_maps = []
    for c in range(NCORES):
        b, t = c // TP, c % TP
        freqs = positions[b].astype(np.float32)[None, :] * inv_freq[:, None]
        cos = np.cos(freqs)                                       # [64, S]
        sin = np.sin(freqs)
        cosf = np.concatenate([cos, cos], axis=0).astype(np.float32)
        sins = np.concatenate([-sin, sin], axis=0).astype(np.float32)

        # hid_r: [quarter, p, kt, 512] with h = kt*128 + p
        hid_r = np.ascontiguousarray(
            hidden_states[b].T.reshape(KT, P, NSG, SG).transpose(2, 1, 0, 3)
        ).astype(bf16)
        # wqk cols for this rank: 8 q heads then 2 kv heads (neox-permuted)
        wq = wq_all[:, 8 * t:8 * t + 8].reshape(H, QC)
        wk = wk_all[:, 2 * t:2 * t + 2].reshape(H, KC)
        wqk = np.concatenate([wq, wk], axis=1)                    # [H, 1280]
        # wqk_r: [ct, p, kt, 128] with h = kt*128 + p, col = ct*128 + i
        wqk_r = np.ascontiguousarray(
            wqk.reshape(KT, P, NCT, P).transpose(2, 1, 0, 3)
        ).astype(bf16)
        wv = wv_all[:, 2 * t:2 * t + 2].reshape(H, KC)
        wv_r = np.ascontiguousarray(
            wv.reshape(KT, P, KC).transpose(1, 0, 2)
        ).astype(bf16)
        # wo rows ordered to match half-group gathers: o_proj k-tile
        # kt = half*16 + r*4 + h2 holds global head 8r + 4*half + h2.
        head_order = np.array(
            [8 * r + 4 * half + h2
             for half in range(2) for r in range(TP) for h2 in range(4)]
        )
        wo_r = np.ascontiguousarray(
            w_o.reshape(NQ, D, H)[head_order, :, OC * t:OC * (t + 1)]
            .transpose(1, 0, 2)
        ).astype(bf16)
        in_maps.append({
            "hid_r": hid_r,
            "wqk_r": wqk_r,
            "wv_r": wv_r,
            "wo_r": wo_r,
            "cosf": cosf,
            "sins": sins,
        })
    return in_maps


_NC_CACHE = {}


def kernel(hidden_states, w_qkv, w_o, positions, _trace=False):
    if "nc" not in _NC_CACHE:
        _NC_CACHE["nc"] = build_program()
    nc = _NC_CACHE["nc"]
    in_maps = _prep_inputs(hidden_states, w_qkv, w_o, positions)
    res = run_bass_kernel_spmd(nc, in_maps, list(range(NCORES)), trace=_trace)
    out_full = np.empty((B, S, H), dtype=np.float32)
    for c in range(NCORES):
        b, t = c // TP, c % TP
        out_full[b, :, OC * t:OC * (t + 1)] = res.results[c]["out"]
    if _trace:
        kernel.last_exec_time_ns = res.exec_time_ns
        kernel.last_profile = res
    return out_full
